# revision 2
# baseline (speedup 1.0000x reference)
"""CollapseEngine (6-layer anchored collapse dynamics) — Trainium2 Bass kernel.

Self-contained: builds a Bass/Tile kernel, shards the batch across 8
NeuronCores (data-parallel, weights/anchors replicated), runs via
run_bass_kernel_spmd, and reassembles full outputs.

Device kernel layout (per core, 16384 rows):
  - row-major state tiles [128 rows, 256 feat]; f32r matmuls on the PE
    (TF32-class precision at full 1 cycle/row rate);
  - the per-row MLP runs as PE matmuls with the transposed h tile as the
    stationary operand; anchor dot products ride in the same matmul as 3
    extra output columns;
  - the anchor force term and the per-row h-scale (1 - 0.1*s and the norm
    clip factor folded together) are injected into the same PSUM
    accumulation as the W2 matmul via a K=4 matmul and a diagonal matmul;
  - state h is kept UNCLIPPED; the clip factor f of the previous layer is
    folded into the tanh scale, the stats corrections, and the diag matmul
    (so the clip multiply costs no extra full-size pass);
  - per-row scalar math (norms, aligns, force coefficients, clip factors)
    is batched across 16 tiles per instruction on the vector engine.
"""
import sys

if "/opt/trn_rl_repo" not in sys.path:
    sys.path.insert(0, "/opt/trn_rl_repo")

import numpy as np
import concourse.bass as bass  # noqa: F401
import concourse.mybir as mybir
from concourse import bacc
from concourse.tile import TileContext
from concourse.bass_utils import run_bass_kernel_spmd

F32 = mybir.dt.float32
F32R = mybir.dt.float32r
AF = mybir.ActivationFunctionType
ALU = mybir.AluOpType
AX = mybir.AxisListType

D = 256
P = 128
NCOL = 260          # 256 z cols + 3 dot cols + 1 pad (f32r even-size restriction)
HG = 2              # tiles per half-group (psum granularity)
N_CORES = 8
NT = 128            # row tiles of 128 per core: 8 * 128 * 128 = 131072 rows
SG = 16             # tiles per supergroup (stats batching granularity)


def _build(nt=NT, sg=SG):
    assert nt % sg == 0 and sg % 4 == 0
    nsg = nt // sg
    qg = 4                      # tiles per phase-A quarter-group
    nqg = sg // qg
    half = sg // 2
    nhg = sg // HG

    nc = bacc.Bacc(None, target_bir_lowering=False)

    hin = nc.dram_tensor("hin", [nt * P, D], F32, kind="ExternalInput")
    nsq0 = nc.dram_tensor("nsq0", [P, nt], F32, kind="ExternalInput")
    rhs1 = nc.dram_tensor("rhs1", [2, P, NCOL], F32, kind="ExternalInput")
    rhs2 = nc.dram_tensor("rhs2", [2, P, D], F32, kind="ExternalInput")
    fmat = nc.dram_tensor("fmat", [4, D], F32, kind="ExternalInput")
    mask01 = nc.dram_tensor("mask01", [P, P], F32, kind="ExternalInput")
    hout = nc.dram_tensor("hout", [nt * P, D], F32, kind="ExternalOutput")
    trout = nc.dram_tensor("trout", [P, nt, 54], F32, kind="ExternalOutput")

    hin_t = hin.rearrange("(t p) d -> p t d", p=P)
    hout_t = hout.rearrange("(t p) d -> p t d", p=P)

    with TileContext(nc) as tc:
        with (
            tc.tile_pool(name="consts", bufs=1) as consts,
            tc.tile_pool(name="state", bufs=1) as state,
            tc.tile_pool(name="ldst", bufs=1) as ldst,
            tc.tile_pool(name="work", bufs=2) as work,
            tc.tile_pool(name="stats", bufs=2) as stats,
            tc.tile_pool(name="zps", bufs=1, space="PSUM") as zps,
            tc.tile_pool(name="dps", bufs=2, space="PSUM") as dps,
            tc.tile_pool(name="tps", bufs=1, space="PSUM") as tps,
        ):
            # ---- constants ----
            c_rhs1_32 = consts.tile([P, 2, NCOL], F32)
            c_rhs2_32 = consts.tile([P, 2, D], F32)
            c_fmat_32 = consts.tile([4, D], F32)
            c_mask = consts.tile([P, P], F32)
            nc.sync.dma_start(out=c_rhs1_32, in_=rhs1.rearrange("k p c -> p k c"))
            nc.sync.dma_start(out=c_rhs2_32, in_=rhs2.rearrange("k p c -> p k c"))
            nc.sync.dma_start(out=c_fmat_32, in_=fmat[:, :])
            nc.sync.dma_start(out=c_mask, in_=mask01[:, :])
            c_rhs1 = consts.tile([P, 2, NCOL], F32R)
            c_rhs2 = consts.tile([P, 2, D], F32R)
            c_fmat = consts.tile([4, D], F32R)
            c_id = consts.tile([P, P], F32R)      # identity for f32r transposes
            c_id32 = consts.tile([P, P], F32)     # identity for fp32 transposes
            c_ones = consts.tile([P, 1], F32)
            nc.vector.tensor_copy(out=c_rhs1, in_=c_rhs1_32)
            nc.vector.tensor_copy(out=c_rhs2, in_=c_rhs2_32)
            nc.vector.tensor_copy(out=c_fmat, in_=c_fmat_32)
            nc.vector.tensor_copy(out=c_id, in_=c_mask)
            nc.vector.tensor_copy(out=c_id32, in_=c_mask)
            nc.vector.memset(c_ones, 1.0)

            for isg in range(nsg):
                t0 = isg * sg
                h_state = state.tile([P, sg, D], F32R, tag="h_state")
                h32 = ldst.tile([P, sg, D], F32, tag="h32")
                nc.sync.dma_start(out=h32, in_=hin_t[:, t0:t0 + sg, :])
                nc.vector.tensor_copy(out=h_state, in_=h32)
                nsq2_sb = state.tile([P, sg], F32, tag="nsq2")
                nc.sync.dma_start(out=nsq2_sb, in_=nsq0[:, t0:t0 + sg])
                dots_sb = state.tile([P, sg, 3], F32, tag="dots")
                tr_sb = state.tile([P, sg, 54], F32, tag="traces")
                fprev = state.tile([P, sg], F32, tag="fprev")

                for layer in range(6):
                    first = layer == 0
                    # ---- fprev = min(1, 10/(||h||+1e-8)); nsq_true ----
                    if first:
                        nc.vector.memset(fprev, 1.0)
                        nsq_true = nsq2_sb
                    else:
                        st_n = stats.tile([P, sg], F32, tag="st_n")
                        nc.scalar.activation(out=st_n, in_=nsq2_sb, func=AF.Sqrt)
                        st_rc = stats.tile([P, sg], F32, tag="st_rc")
                        nc.vector.tensor_scalar_add(
                            out=st_rc, in0=st_n, scalar1=1e-8)
                        nc.vector.reciprocal(out=st_rc, in_=st_rc)
                        nc.vector.tensor_scalar(
                            out=fprev, in0=st_rc, scalar1=10.0, scalar2=1.0,
                            op0=ALU.mult, op1=ALU.min)
                        nsq_true = stats.tile([P, sg], F32, tag="st_nsqt")
                        nc.vector.tensor_tensor(
                            out=nsq_true, in0=fprev, in1=fprev, op=ALU.mult)
                        nc.vector.tensor_tensor(
                            out=nsq_true, in0=nsq_true, in1=nsq2_sb, op=ALU.mult)

                    t1T_all = state.tile([P, sg, 2, P], F32R, tag="t1T")
                    cpack = work.tile([P, sg, 4], F32R, tag="cpack")
                    diag = work.tile([P, sg, P], F32R, tag="diag")

                    def phase_a(iqg):
                        tb = iqg * qg
                        pt = tps.tile([P, qg, 2, P], F32R, tag="ptrans")
                        for i in range(qg):
                            for k in range(2):
                                nc.tensor.transpose(
                                    pt[:, i, k, :],
                                    h_state[:, tb + i, k * P:(k + 1) * P], c_id)
                        hT = work.tile([P, qg, 2, P], F32R, tag="hT")
                        nc.vector.tensor_copy(out=hT, in_=pt)
                        pz = zps.tile([P, qg, 512], F32, tag="pz")
                        for i in range(qg):
                            for k in range(2):
                                nc.tensor.matmul(
                                    pz[:, i, 0:NCOL], hT[:, i, k, :],
                                    c_rhs1[:, k, :], start=(k == 0), stop=(k == 1))
                        t1 = work.tile([P, qg, D], F32, tag="t1")
                        if first:
                            nc.scalar.activation(
                                out=t1, in_=pz[:, :, 0:D], func=AF.Tanh)
                        else:
                            for i in range(qg):
                                nc.scalar.activation(
                                    out=t1[:, i, :], in_=pz[:, i, 0:D],
                                    func=AF.Tanh,
                                    scale=fprev[:, tb + i:tb + i + 1])
                        nc.vector.tensor_copy(
                            out=dots_sb[:, tb:tb + qg, :], in_=pz[:, :, D:D + 3])
                        pt2 = tps.tile([P, qg, 2, P], F32, tag="ptrans")
                        for i in range(qg):
                            for k in range(2):
                                nc.tensor.transpose(
                                    pt2[:, i, k, :],
                                    t1[:, i, k * P:(k + 1) * P], c_id32)
                        nc.vector.tensor_copy(
                            out=t1T_all[:, tb:tb + qg, :, :],
                            in_=pt2.bitcast(F32R))

                    def do_stats(hb):
                        sl = slice(hb, hb + half)
                        st_dt = stats.tile([P, half, 3], F32, tag="st_dt")
                        if first:
                            nc.vector.tensor_copy(out=st_dt, in_=dots_sb[:, sl, :])
                        else:
                            nc.vector.tensor_tensor(
                                out=st_dt, in0=dots_sb[:, sl, :],
                                in1=fprev[:, sl].unsqueeze(-1).to_broadcast(
                                    [P, half, 3]),
                                op=ALU.mult)
                        nst = nsq_true[:, sl]
                        st_iv = stats.tile([P, half], F32, tag="st_iv")
                        nc.scalar.activation(out=st_iv, in_=nst, func=AF.Sqrt)
                        nc.vector.tensor_scalar_max(
                            out=st_iv, in0=st_iv, scalar1=1e-12)
                        nc.vector.reciprocal(out=st_iv, in_=st_iv)
                        tr_al = tr_sb[:, sl, layer * 9 + 0:layer * 9 + 3]
                        tr_dv = tr_sb[:, sl, layer * 9 + 3:layer * 9 + 6]
                        tr_tn = tr_sb[:, sl, layer * 9 + 6:layer * 9 + 9]
                        nc.vector.tensor_tensor(
                            out=tr_al, in0=st_dt,
                            in1=st_iv.unsqueeze(-1).to_broadcast([P, half, 3]),
                            op=ALU.mult)
                        nc.vector.tensor_scalar(
                            out=tr_dv, in0=tr_al, scalar1=-1.0, scalar2=1.0,
                            op0=ALU.mult, op1=ALU.add)
                        nc.scalar.activation(out=tr_tn, in_=tr_dv, func=AF.Abs)
                        st_r2 = stats.tile([P, half, 3], F32, tag="st_r2")
                        nc.vector.tensor_scalar(
                            out=st_r2, in0=st_dt, scalar1=-2.0, scalar2=1.0,
                            op0=ALU.mult, op1=ALU.add)
                        nc.vector.tensor_tensor(
                            out=st_r2, in0=st_r2,
                            in1=nst.unsqueeze(-1).to_broadcast([P, half, 3]),
                            op=ALU.add)
                        nc.vector.tensor_scalar_max(
                            out=st_r2, in0=st_r2, scalar1=1e-24)
                        nc.scalar.activation(out=st_r2, in_=st_r2, func=AF.Sqrt)
                        nc.vector.reciprocal(out=st_r2, in_=st_r2)
                        st_c = stats.tile([P, half, 3], F32, tag="st_c")
                        nc.vector.tensor_tensor(
                            out=st_c, in0=tr_dv, in1=st_r2, op=ALU.mult)
                        st_s = stats.tile([P, half], F32, tag="st_s")
                        nc.vector.reduce_sum(out=st_s, in_=st_c, axis=AX.X)
                        nc.vector.tensor_copy(out=cpack[:, sl, 0:3], in_=st_c)
                        nc.vector.tensor_copy(
                            out=cpack[:, sl, 3:4],
                            in_=c_ones.unsqueeze(1).to_broadcast([P, half, 1]))
                        st_g = stats.tile([P, half], F32, tag="st_g")
                        nc.vector.tensor_scalar(
                            out=st_g, in0=st_s, scalar1=-0.1, scalar2=1.0,
                            op0=ALU.mult, op1=ALU.add)
                        if not first:
                            nc.vector.tensor_tensor(
                                out=st_g, in0=st_g, in1=fprev[:, sl],
                                op=ALU.mult)
                        nc.vector.tensor_tensor(
                            out=diag[:, sl, :],
                            in0=c_mask.unsqueeze(1).to_broadcast([P, half, P]),
                            in1=st_g.unsqueeze(-1).to_broadcast([P, half, P]),
                            op=ALU.mult)

                    def phase_c(ihg):
                        tb = ihg * HG
                        pc = tps.tile([4, HG, P], F32R, tag="ptrans")
                        for i in range(HG):
                            nc.tensor.transpose(
                                pc[:, i, :], cpack[:, tb + i, :], c_id)
                        cT = work.tile([4, HG, P], F32R, tag="cT")
                        nc.vector.tensor_copy(out=cT, in_=pc)
                        pdt = dps.tile([P, HG, D], F32, tag="pd")
                        for i in range(HG):
                            for k in range(2):
                                nc.tensor.matmul(
                                    pdt[:, i, :], t1T_all[:, tb + i, k, :],
                                    c_rhs2[:, k, :], start=(k == 0), stop=False)
                            nc.tensor.matmul(
                                pdt[:, i, :], cT[:, i, :], c_fmat,
                                start=False, stop=False)
                            nc.tensor.matmul(
                                pdt[:, i, :], diag[:, tb + i, :],
                                h_state[:, tb + i, :], start=False, stop=True)
                        sqscr = work.tile([P, HG, D], F32, tag="sqscr")
                        for i in range(HG):
                            nc.scalar.activation(
                                out=sqscr[:, i, :], in_=pdt[:, i, :],
                                func=AF.Square,
                                accum_out=nsq2_sb[:, tb + i:tb + i + 1])
                        nc.vector.tensor_copy(
                            out=h_state[:, tb:tb + HG, :], in_=pdt)

                    # emission order: A for half 0, stats(half 0) overlaps A of
                    # half 1 on DVE, then C(half 0) overlaps stats(half 1).
                    for iqg in range(nqg // 2):
                        phase_a(iqg)
                    do_stats(0)
                    for iqg in range(nqg // 2, nqg):
                        phase_a(iqg)
                    do_stats(half)
                    for ihg in range(nhg):
                        phase_c(ihg)

                # ---- final clip + store ----
                st_n = stats.tile([P, sg], F32, tag="st_n")
                nc.scalar.activation(out=st_n, in_=nsq2_sb, func=AF.Sqrt)
                st_rc = stats.tile([P, sg], F32, tag="st_rc")
                nc.vector.tensor_scalar_add(out=st_rc, in0=st_n, scalar1=1e-8)
                nc.vector.reciprocal(out=st_rc, in_=st_rc)
                ffin = stats.tile([P, sg], F32, tag="st_ffin")
                nc.vector.tensor_scalar(
                    out=ffin, in0=st_rc, scalar1=10.0, scalar2=1.0,
                    op0=ALU.mult, op1=ALU.min)
                hfin = ldst.tile([P, sg, D], F32, tag="hfin")
                nc.vector.tensor_tensor(
                    out=hfin, in0=h_state.bitcast(F32),
                    in1=ffin.unsqueeze(-1).to_broadcast([P, sg, D]),
                    op=ALU.mult)
                nc.sync.dma_start(out=hout_t[:, t0:t0 + sg, :], in_=hfin)
                nc.sync.dma_start(out=trout[:, t0:t0 + sg, :], in_=tr_sb)

    nc.compile()
    return nc


def _host_inputs(h0, W1, b1, W2, b2, anchors, nt=NT, n_cores=N_CORES):
    B, Dd = h0.shape
    assert Dd == D and B == nt * P * n_cores
    an = anchors / np.maximum(
        np.linalg.norm(anchors, axis=-1, keepdims=True), 1e-12)
    an = an.astype(np.float32)
    assert np.abs(b1).max() == 0.0, "kernel assumes b1 == 0 (spec fill=zeros)"
    rhs1 = np.zeros((2, P, NCOL), np.float32)
    rhs2 = np.zeros((2, P, D), np.float32)
    W1T = np.ascontiguousarray(W1.T)
    W2T = np.ascontiguousarray(W2.T)
    anT = np.ascontiguousarray(an.T)
    for k in range(2):
        rhs1[k, :, 0:D] = W1T[k * P:(k + 1) * P, :]
        rhs1[k, :, D:D + 3] = anT[k * P:(k + 1) * P, :]
        rhs2[k] = W2T[k * P:(k + 1) * P, :]
    fmat = np.zeros((4, D), np.float32)
    fmat[0:3] = 0.1 * an
    fmat[3] = b2
    mask01 = np.eye(P, dtype=np.float32)
    rows_core = nt * P
    nsq = (h0.astype(np.float64) ** 2).sum(-1).astype(np.float32)
    in_maps = []
    for c in range(n_cores):
        sl = slice(c * rows_core, (c + 1) * rows_core)
        in_maps.append({
            "hin": np.ascontiguousarray(h0[sl]),
            "nsq0": np.ascontiguousarray(nsq[sl].reshape(nt, P).T),
            "rhs1": rhs1, "rhs2": rhs2, "fmat": fmat, "mask01": mask01,
        })
    return in_maps


def _assemble(results, nt=NT, n_cores=N_CORES):
    rows_core = nt * P
    h_final = np.concatenate([np.asarray(r["hout"]) for r in results], axis=0)
    tr = np.stack([np.asarray(r["trout"]) for r in results])  # [C, P, nt, 54]
    tr = tr.transpose(0, 2, 1, 3).reshape(n_cores * rows_core, 54)
    tr = tr.reshape(-1, 6, 3, 3)                  # [B, layer, kind, anchor]
    aligns = np.ascontiguousarray(tr[:, :, 0, :].transpose(1, 0, 2))
    divs = np.ascontiguousarray(tr[:, :, 1, :].transpose(1, 0, 2))
    tens = np.ascontiguousarray(tr[:, :, 2, :].transpose(1, 0, 2))
    return h_final, aligns, divs, tens


_NC_CACHE = {}


def kernel(h0, W1, b1, W2, b2, anchors):
    h0 = np.asarray(h0, np.float32)
    W1 = np.asarray(W1, np.float32)
    b1 = np.asarray(b1, np.float32)
    W2 = np.asarray(W2, np.float32)
    b2 = np.asarray(b2, np.float32)
    anchors = np.asarray(anchors, np.float32)
    if "nc" not in _NC_CACHE:
        _NC_CACHE["nc"] = _build()
    nc = _NC_CACHE["nc"]
    in_maps = _host_inputs(h0, W1, b1, W2, b2, anchors)
    res = run_bass_kernel_spmd(nc, in_maps, list(range(N_CORES)))
    return _assemble(res.results)


# revision 3
# speedup vs baseline: 1.2742x; 1.2742x over previous
"""CollapseEngine (6-layer anchored collapse dynamics) — Trainium2 Bass kernel.

Self-contained: builds a Bass/Tile kernel, shards the batch across 8
NeuronCores (data-parallel, weights/anchors replicated), runs via
run_bass_kernel_spmd, and reassembles full outputs.

Device kernel layout (per core, 16384 rows):
  - row-major state tiles [128 rows, 256 feat]; f32r matmuls on the PE
    (TF32-class precision at full 1 cycle/row rate);
  - the per-row MLP runs as PE matmuls with the transposed h tile as the
    stationary operand; anchor dot products ride in the same matmul as 3
    extra output columns;
  - the anchor force term and the per-row h-scale (1 - 0.1*s and the norm
    clip factor folded together) are injected into the same PSUM
    accumulation as the W2 matmul via a K=4 matmul and a diagonal matmul;
  - state h is kept UNCLIPPED; the clip factor f of the previous layer is
    folded into the tanh scale, the stats corrections, and the diag matmul
    (so the clip multiply costs no extra full-size pass);
  - per-row scalar math (norms, aligns, force coefficients, clip factors)
    is batched across 16 tiles per instruction on the vector engine.
"""
import sys

if "/opt/trn_rl_repo" not in sys.path:
    sys.path.insert(0, "/opt/trn_rl_repo")

import numpy as np
import concourse.bass as bass  # noqa: F401
import concourse.mybir as mybir
from concourse import bacc
from concourse.tile import TileContext
from concourse.bass_utils import run_bass_kernel_spmd

F32 = mybir.dt.float32
F32R = mybir.dt.float32r
AF = mybir.ActivationFunctionType
ALU = mybir.AluOpType
AX = mybir.AxisListType

D = 256
P = 128
NCOL = 260          # 256 z cols + 3 dot cols + 1 pad (f32r even-size restriction)
HG = 2              # tiles per half-group (psum granularity)
N_CORES = 8
NT = 128            # row tiles of 128 per core: 8 * 128 * 128 = 131072 rows
SG = 16             # tiles per supergroup (stats batching granularity)


def _build(nt=NT, sg=SG):
    assert nt % sg == 0 and sg % 4 == 0
    nsg = nt // sg
    qg = 2                      # tiles per phase-A group
    nqg = sg // qg
    half = sg // 2
    nhg = sg // HG

    nc = bacc.Bacc(None, target_bir_lowering=False)

    hin = nc.dram_tensor("hin", [nt * P, D], F32, kind="ExternalInput")
    nsq0 = nc.dram_tensor("nsq0", [P, nt], F32, kind="ExternalInput")
    rhs1 = nc.dram_tensor("rhs1", [2, P, NCOL], F32, kind="ExternalInput")
    rhs2 = nc.dram_tensor("rhs2", [2, P, D], F32, kind="ExternalInput")
    fmat = nc.dram_tensor("fmat", [4, D], F32, kind="ExternalInput")
    mask01 = nc.dram_tensor("mask01", [P, P], F32, kind="ExternalInput")
    hout = nc.dram_tensor("hout", [nt * P, D], F32, kind="ExternalOutput")
    trout = nc.dram_tensor("trout", [P, nt, 54], F32, kind="ExternalOutput")

    hin_t = hin.rearrange("(t p) d -> p t d", p=P)
    hout_t = hout.rearrange("(t p) d -> p t d", p=P)

    with TileContext(nc) as tc:
        with (
            tc.tile_pool(name="consts", bufs=1) as consts,
            tc.tile_pool(name="state", bufs=1) as state,
            tc.tile_pool(name="ldst", bufs=1) as ldst,
            tc.tile_pool(name="work", bufs=2) as work,
            tc.tile_pool(name="stats", bufs=2) as stats,
            tc.tile_pool(name="zps", bufs=2, space="PSUM") as zps,
            tc.tile_pool(name="dps", bufs=2, space="PSUM") as dps,
            tc.tile_pool(name="tps", bufs=1, space="PSUM") as tps,
        ):
            # ---- constants ----
            c_rhs1_32 = consts.tile([P, 2, NCOL], F32)
            c_rhs2_32 = consts.tile([P, 2, D], F32)
            c_fmat_32 = consts.tile([4, D], F32)
            c_mask = consts.tile([P, P], F32)
            nc.sync.dma_start(out=c_rhs1_32, in_=rhs1.rearrange("k p c -> p k c"))
            nc.sync.dma_start(out=c_rhs2_32, in_=rhs2.rearrange("k p c -> p k c"))
            nc.sync.dma_start(out=c_fmat_32, in_=fmat[:, :])
            nc.sync.dma_start(out=c_mask, in_=mask01[:, :])
            c_rhs1 = consts.tile([P, 2, NCOL], F32R)
            c_rhs2 = consts.tile([P, 2, D], F32R)
            c_fmat = consts.tile([4, D], F32R)
            c_id = consts.tile([P, P], F32R)      # identity for f32r transposes
            c_id32 = consts.tile([P, P], F32)     # identity for fp32 transposes
            c_ones = consts.tile([P, 1], F32)
            nc.vector.tensor_copy(out=c_rhs1, in_=c_rhs1_32)
            nc.vector.tensor_copy(out=c_rhs2, in_=c_rhs2_32)
            nc.vector.tensor_copy(out=c_fmat, in_=c_fmat_32)
            nc.vector.tensor_copy(out=c_id, in_=c_mask)
            nc.vector.tensor_copy(out=c_id32, in_=c_mask)
            nc.vector.memset(c_ones, 1.0)

            for isg in range(nsg):
                t0 = isg * sg
                h_state = state.tile([P, sg, D], F32R, tag="h_state")
                h32 = ldst.tile([P, sg, D], F32, tag="h32")
                nc.sync.dma_start(out=h32, in_=hin_t[:, t0:t0 + sg, :])
                nc.vector.tensor_copy(out=h_state, in_=h32)
                nsq2_sb = state.tile([P, sg], F32, tag="nsq2")
                nc.sync.dma_start(out=nsq2_sb, in_=nsq0[:, t0:t0 + sg])
                dots_sb = state.tile([P, sg, 3], F32, tag="dots")
                tr_sb = state.tile([P, sg, 54], F32, tag="traces")
                fprev = state.tile([P, sg], F32, tag="fprev")

                for layer in range(6):
                    first = layer == 0
                    # ---- fprev = min(1, 10/(||h||+1e-8)); nsq_true ----
                    if first:
                        nc.vector.memset(fprev, 1.0)
                        nsq_true = nsq2_sb
                    else:
                        st_n = stats.tile([P, sg], F32, tag="st_n")
                        nc.scalar.activation(out=st_n, in_=nsq2_sb, func=AF.Sqrt)
                        st_rc = stats.tile([P, sg], F32, tag="st_rc")
                        nc.vector.tensor_scalar_add(
                            out=st_rc, in0=st_n, scalar1=1e-8)
                        nc.vector.reciprocal(out=st_rc, in_=st_rc)
                        nc.vector.tensor_scalar(
                            out=fprev, in0=st_rc, scalar1=10.0, scalar2=1.0,
                            op0=ALU.mult, op1=ALU.min)
                        nsq_true = stats.tile([P, sg], F32, tag="st_nsqt")
                        nc.vector.tensor_tensor(
                            out=nsq_true, in0=fprev, in1=fprev, op=ALU.mult)
                        nc.vector.tensor_tensor(
                            out=nsq_true, in0=nsq_true, in1=nsq2_sb, op=ALU.mult)

                    t1T_all = state.tile([P, sg, 2, P], F32R, tag="t1T")
                    cpack = work.tile([P, sg, 4], F32R, tag="cpack")
                    diag = work.tile([P, sg, P], F32R, tag="diag")

                    def phase_a(iqg):
                        tb = iqg * qg
                        pt = tps.tile([P, qg, 2, P], F32R, tag="ptrans")
                        for i in range(qg):
                            for k in range(2):
                                nc.tensor.transpose(
                                    pt[:, i, k, :],
                                    h_state[:, tb + i, k * P:(k + 1) * P], c_id)
                        hT = work.tile([P, qg, 2, P], F32R, tag="hT")
                        nc.vector.tensor_copy(out=hT, in_=pt)
                        pz = zps.tile([P, qg, 512], F32, tag="pz")
                        for i in range(qg):
                            for k in range(2):
                                nc.tensor.matmul(
                                    pz[:, i, 0:NCOL], hT[:, i, k, :],
                                    c_rhs1[:, k, :], start=(k == 0), stop=(k == 1))
                        t1 = work.tile([P, qg, D], F32, tag="t1")
                        if first:
                            nc.scalar.activation(
                                out=t1, in_=pz[:, :, 0:D], func=AF.Tanh)
                        else:
                            for i in range(qg):
                                nc.scalar.activation(
                                    out=t1[:, i, :], in_=pz[:, i, 0:D],
                                    func=AF.Tanh,
                                    scale=fprev[:, tb + i:tb + i + 1])
                        nc.vector.tensor_copy(
                            out=dots_sb[:, tb:tb + qg, :], in_=pz[:, :, D:D + 3])
                        pt2 = tps.tile([P, qg, 2, P], F32, tag="ptrans2")
                        for i in range(qg):
                            for k in range(2):
                                nc.tensor.transpose(
                                    pt2[:, i, k, :],
                                    t1[:, i, k * P:(k + 1) * P], c_id32)
                        nc.vector.tensor_copy(
                            out=t1T_all[:, tb:tb + qg, :, :],
                            in_=pt2.bitcast(F32R))

                    def do_stats(hb):
                        sl = slice(hb, hb + half)
                        st_dt = stats.tile([P, half, 3], F32, tag="st_dt")
                        if first:
                            nc.vector.tensor_copy(out=st_dt, in_=dots_sb[:, sl, :])
                        else:
                            nc.vector.tensor_tensor(
                                out=st_dt, in0=dots_sb[:, sl, :],
                                in1=fprev[:, sl].unsqueeze(-1).to_broadcast(
                                    [P, half, 3]),
                                op=ALU.mult)
                        nst = nsq_true[:, sl]
                        st_iv = stats.tile([P, half], F32, tag="st_iv")
                        nc.scalar.activation(out=st_iv, in_=nst, func=AF.Sqrt)
                        nc.vector.tensor_scalar_max(
                            out=st_iv, in0=st_iv, scalar1=1e-12)
                        nc.vector.reciprocal(out=st_iv, in_=st_iv)
                        tr_al = tr_sb[:, sl, layer * 9 + 0:layer * 9 + 3]
                        tr_dv = tr_sb[:, sl, layer * 9 + 3:layer * 9 + 6]
                        tr_tn = tr_sb[:, sl, layer * 9 + 6:layer * 9 + 9]
                        nc.vector.tensor_tensor(
                            out=tr_al, in0=st_dt,
                            in1=st_iv.unsqueeze(-1).to_broadcast([P, half, 3]),
                            op=ALU.mult)
                        nc.vector.tensor_scalar(
                            out=tr_dv, in0=tr_al, scalar1=-1.0, scalar2=1.0,
                            op0=ALU.mult, op1=ALU.add)
                        nc.scalar.activation(out=tr_tn, in_=tr_dv, func=AF.Abs)
                        st_r2 = stats.tile([P, half, 3], F32, tag="st_r2")
                        nc.vector.tensor_scalar(
                            out=st_r2, in0=st_dt, scalar1=-2.0, scalar2=1.0,
                            op0=ALU.mult, op1=ALU.add)
                        nc.vector.tensor_tensor(
                            out=st_r2, in0=st_r2,
                            in1=nst.unsqueeze(-1).to_broadcast([P, half, 3]),
                            op=ALU.add)
                        nc.vector.tensor_scalar_max(
                            out=st_r2, in0=st_r2, scalar1=1e-24)
                        nc.scalar.activation(out=st_r2, in_=st_r2, func=AF.Sqrt)
                        nc.vector.reciprocal(out=st_r2, in_=st_r2)
                        st_c = stats.tile([P, half, 3], F32, tag="st_c")
                        nc.vector.tensor_tensor(
                            out=st_c, in0=tr_dv, in1=st_r2, op=ALU.mult)
                        st_s = stats.tile([P, half], F32, tag="st_s")
                        nc.vector.reduce_sum(out=st_s, in_=st_c, axis=AX.X)
                        nc.vector.tensor_copy(out=cpack[:, sl, 0:3], in_=st_c)
                        nc.vector.tensor_copy(
                            out=cpack[:, sl, 3:4],
                            in_=c_ones.unsqueeze(1).to_broadcast([P, half, 1]))
                        st_g = stats.tile([P, half], F32, tag="st_g")
                        nc.vector.tensor_scalar(
                            out=st_g, in0=st_s, scalar1=-0.1, scalar2=1.0,
                            op0=ALU.mult, op1=ALU.add)
                        if not first:
                            nc.vector.tensor_tensor(
                                out=st_g, in0=st_g, in1=fprev[:, sl],
                                op=ALU.mult)
                        nc.vector.tensor_tensor(
                            out=diag[:, sl, :],
                            in0=c_mask.unsqueeze(1).to_broadcast([P, half, P]),
                            in1=st_g.unsqueeze(-1).to_broadcast([P, half, P]),
                            op=ALU.mult)

                    def phase_c(ihg):
                        tb = ihg * HG
                        pc = tps.tile([4, HG, P], F32R, tag="ptrans2")
                        for i in range(HG):
                            nc.tensor.transpose(
                                pc[:, i, :], cpack[:, tb + i, :], c_id)
                        cT = work.tile([4, HG, P], F32R, tag="cT")
                        nc.vector.tensor_copy(out=cT, in_=pc)
                        pdt = dps.tile([P, HG, D], F32, tag="pd")
                        for i in range(HG):
                            for k in range(2):
                                nc.tensor.matmul(
                                    pdt[:, i, :], t1T_all[:, tb + i, k, :],
                                    c_rhs2[:, k, :], start=(k == 0), stop=False)
                            nc.tensor.matmul(
                                pdt[:, i, :], cT[:, i, :], c_fmat,
                                start=False, stop=False)
                            nc.tensor.matmul(
                                pdt[:, i, :], diag[:, tb + i, :],
                                h_state[:, tb + i, :], start=False, stop=True)
                        sqscr = work.tile([P, HG, D], F32, tag="sqscr")
                        for i in range(HG):
                            nc.scalar.activation(
                                out=sqscr[:, i, :], in_=pdt[:, i, :],
                                func=AF.Square,
                                accum_out=nsq2_sb[:, tb + i:tb + i + 1])
                        nc.vector.tensor_copy(
                            out=h_state[:, tb:tb + HG, :], in_=pdt)

                    # emission order: A for half 0, stats(half 0) overlaps A of
                    # half 1 on DVE, then C(half 0) overlaps stats(half 1).
                    for iqg in range(nqg // 2):
                        phase_a(iqg)
                    do_stats(0)
                    for iqg in range(nqg // 2, nqg):
                        phase_a(iqg)
                    do_stats(half)
                    for ihg in range(nhg):
                        phase_c(ihg)

                # ---- final clip + store ----
                st_n = stats.tile([P, sg], F32, tag="st_n")
                nc.scalar.activation(out=st_n, in_=nsq2_sb, func=AF.Sqrt)
                st_rc = stats.tile([P, sg], F32, tag="st_rc")
                nc.vector.tensor_scalar_add(out=st_rc, in0=st_n, scalar1=1e-8)
                nc.vector.reciprocal(out=st_rc, in_=st_rc)
                ffin = stats.tile([P, sg], F32, tag="st_ffin")
                nc.vector.tensor_scalar(
                    out=ffin, in0=st_rc, scalar1=10.0, scalar2=1.0,
                    op0=ALU.mult, op1=ALU.min)
                hfin = ldst.tile([P, sg, D], F32, tag="hfin")
                nc.vector.tensor_tensor(
                    out=hfin, in0=h_state.bitcast(F32),
                    in1=ffin.unsqueeze(-1).to_broadcast([P, sg, D]),
                    op=ALU.mult)
                nc.sync.dma_start(out=hout_t[:, t0:t0 + sg, :], in_=hfin)
                nc.sync.dma_start(out=trout[:, t0:t0 + sg, :], in_=tr_sb)

    nc.compile()
    return nc


def _host_inputs(h0, W1, b1, W2, b2, anchors, nt=NT, n_cores=N_CORES):
    B, Dd = h0.shape
    assert Dd == D and B == nt * P * n_cores
    an = anchors / np.maximum(
        np.linalg.norm(anchors, axis=-1, keepdims=True), 1e-12)
    an = an.astype(np.float32)
    assert np.abs(b1).max() == 0.0, "kernel assumes b1 == 0 (spec fill=zeros)"
    rhs1 = np.zeros((2, P, NCOL), np.float32)
    rhs2 = np.zeros((2, P, D), np.float32)
    W1T = np.ascontiguousarray(W1.T)
    W2T = np.ascontiguousarray(W2.T)
    anT = np.ascontiguousarray(an.T)
    for k in range(2):
        rhs1[k, :, 0:D] = W1T[k * P:(k + 1) * P, :]
        rhs1[k, :, D:D + 3] = anT[k * P:(k + 1) * P, :]
        rhs2[k] = W2T[k * P:(k + 1) * P, :]
    fmat = np.zeros((4, D), np.float32)
    fmat[0:3] = 0.1 * an
    fmat[3] = b2
    mask01 = np.eye(P, dtype=np.float32)
    rows_core = nt * P
    nsq = (h0.astype(np.float64) ** 2).sum(-1).astype(np.float32)
    in_maps = []
    for c in range(n_cores):
        sl = slice(c * rows_core, (c + 1) * rows_core)
        in_maps.append({
            "hin": np.ascontiguousarray(h0[sl]),
            "nsq0": np.ascontiguousarray(nsq[sl].reshape(nt, P).T),
            "rhs1": rhs1, "rhs2": rhs2, "fmat": fmat, "mask01": mask01,
        })
    return in_maps


def _assemble(results, nt=NT, n_cores=N_CORES):
    rows_core = nt * P
    h_final = np.concatenate([np.asarray(r["hout"]) for r in results], axis=0)
    tr = np.stack([np.asarray(r["trout"]) for r in results])  # [C, P, nt, 54]
    tr = tr.transpose(0, 2, 1, 3).reshape(n_cores * rows_core, 54)
    tr = tr.reshape(-1, 6, 3, 3)                  # [B, layer, kind, anchor]
    aligns = np.ascontiguousarray(tr[:, :, 0, :].transpose(1, 0, 2))
    divs = np.ascontiguousarray(tr[:, :, 1, :].transpose(1, 0, 2))
    tens = np.ascontiguousarray(tr[:, :, 2, :].transpose(1, 0, 2))
    return h_final, aligns, divs, tens


_NC_CACHE = {}


def kernel(h0, W1, b1, W2, b2, anchors):
    h0 = np.asarray(h0, np.float32)
    W1 = np.asarray(W1, np.float32)
    b1 = np.asarray(b1, np.float32)
    W2 = np.asarray(W2, np.float32)
    b2 = np.asarray(b2, np.float32)
    anchors = np.asarray(anchors, np.float32)
    if "nc" not in _NC_CACHE:
        _NC_CACHE["nc"] = _build()
    nc = _NC_CACHE["nc"]
    in_maps = _host_inputs(h0, W1, b1, W2, b2, anchors)
    res = run_bass_kernel_spmd(nc, in_maps, list(range(N_CORES)))
    return _assemble(res.results)
